# revision 8
# baseline (speedup 1.0000x reference)
# Depthwise 4x4 conv (DiagonalwiseRefactorization) on 8 TRN2 NeuronCores.
#
# The mask zeroes every weight except weight[c, c % 64], and with
# feature_group_count=8 the grouped conv collapses to a depthwise conv:
#   out[n, c, ho, wo] = sum_{kh, kw} w[c, kh, kw] * xpad[n, c, ho+kh, wo+kw]
# with pad=1, stride=1: (16, 512, 64, 64) -> (16, 512, 63, 63).
#
# Device strategy (per core: 64 channels x 16 images, no inter-core comm):
#   For each width-tap kw, the H-direction conv is a banded-Toeplitz matmul:
#     out[c, :, wo] += T_c_kw.T @ xrow[c, :, wo+kw]
#   where T_c_kw[h, ho] = w[c, h-ho+1, kw] (64x64, 4 diagonals; the H padding
#   falls out of the band clipping).
#
#   FOUR channels run concurrently in the four 64x64 quadrant tiles of the
#   PE array: a "quad" = 2 channel-pairs; pair0 uses tiles (0,0)/(64,64)
#   into PSUM bank a, pair1 uses (0,64)/(64,0) into bank b (so pair1's two
#   channels land partition-swapped in PSUM; the host unswizzle swaps back).
#   The 4 kw taps accumulate in PSUM per tile.
#
# Precision/traffic plan (DMA-bound kernel; HBM ~358 GB/s per core):
#   - x: bf16, swizzled to [pair, c'*64+h, n, w]            (8.4 MB/core)
#   - Toeplitz: bf16 with the per-channel scale sc_c = 127/(K*norm_c)
#     pre-folded on host, so PSUM values land directly in int8 range;
#     packed into the x DMA stream                            (2.1 MB/core)
#   - out: PSUM fp32 -> int8 copies (RNE, saturating) on ACT/DVE, int8
#     stores; host dequantizes by K*norm_c/127               (4.1 MB/core)
# K=8 gives |psum| <= ~89 < 127 (no clipping) and rel err ~9e-3 < 2e-2.

import sys
import types

import numpy as np
import ml_dtypes

BF16 = ml_dtypes.bfloat16

N_CORES = 8
IMGS = 16
CH_TOT = 512
CH = CH_TOT // N_CORES  # 64 channels per core
PAIRS = CH // 2  # 32
QUADS = PAIRS // 2  # 16
H = W = 64
HO = WO = 63
NHALF = IMGS // 2  # 8 images per psum tile
NFREE = NHALF * WO  # 504 <= 512 (one PSUM bank)
KSAFE = 8.0  # psum int8 range safety factor: bound_c = K * ||w_c||_2


def _install_axon_hooks_shim():
    """Make trace=True work under axon: bass_utils imports
    antenv.axon_hooks, which the container's antenv stub lacks."""
    try:
        import antenv.axon_hooks  # noqa: F401

        return
    except ImportError:
        pass
    try:
        import antenv
    except ImportError:
        return
    mod = types.ModuleType("antenv.axon_hooks")
    mod._hook = None

    def set_axon_ntff_profile_hook(h):
        mod._hook = h

    def get_axon_ntff_profile_hook():
        return mod._hook

    mod.set_axon_ntff_profile_hook = set_axon_ntff_profile_hook
    mod.get_axon_ntff_profile_hook = get_axon_ntff_profile_hook
    sys.modules["antenv.axon_hooks"] = mod
    antenv.axon_hooks = mod
    try:
        from trn_agent_boot.trn_boot import _ntff_profile_via_ctypes

        hook = _ntff_profile_via_ctypes("/opt/axon/libaxon_pjrt.so")
        if hook is not None:
            mod._hook = hook
    except Exception:
        pass


_install_axon_hooks_shim()

import concourse.bacc as bacc  # noqa: E402
import concourse.mybir as mybir  # noqa: E402
import concourse.tile as tile  # noqa: E402
from concourse.bass_utils import run_bass_kernel_spmd  # noqa: E402

LAST_RESULT = None
_NC_CACHE = None


XCOLS = IMGS * W  # 1024
WCOLS = 4 * H  # 256
INCOLS = XCOLS + WCOLS  # 1280 per pair
HCOLS = NHALF * W  # 512 cols per image-half
# quad row layout (bf16 cols): [T_p0|T_p1| xh0_p0|xh0_p1| xh1_p0|xh1_p1]
QCOLS = 2 * INCOLS  # 2560

# Per width-tap kw: x column range [xc0, xc1) and output wo range [wo0, wo1).
# out[wo] += w[.., kw] * x[wo + kw - 1]; clipped where x would be padding.
# kw=1 goes first: it covers the full wo range, so its start=True write sets
# PSUM has_written everywhere before the partial-range taps accumulate.
KW_PLAN = [
    (1, 0, 63, 0, 63),  # kw, xc0, xc1, wo0, wo1
    (2, 1, 64, 0, 63),
    (0, 0, 62, 1, 63),
    (3, 2, 64, 0, 62),
]


def _build_nc():
    # Bass.__init__ emits four [128,1] const-AP memsets on GpSimd whose DMA
    # completion delays the first all-engine barrier; this kernel never reads
    # the const APs (matmul/copy/dma only), so skip those preamble memsets.
    import concourse.bass as bassmod

    orig_memset = bassmod.BassGpSimd.memset
    bassmod.BassGpSimd.memset = lambda self, ap, constant: None
    try:
        nc = bacc.Bacc(
            "TRN2", target_bir_lowering=False, debug=False, num_devices=N_CORES
        )
    finally:
        bassmod.BassGpSimd.memset = orig_memset

    xd = nc.dram_tensor(
        "xin", [QUADS, 128, QCOLS], mybir.dt.bfloat16, kind="ExternalInput"
    )
    od = nc.dram_tensor(
        "out", [QUADS, 128, 2, 2, NFREE], mybir.dt.int8, kind="ExternalOutput"
    )

    with tile.TileContext(nc) as tc:
        with (
            tc.tile_pool(name="xp", bufs=10) as xp,
            tc.tile_pool(name="ps", bufs=2, space="PSUM") as ps,
            tc.tile_pool(name="op", bufs=10) as op,
        ):
            # Warm up the PE HAM clock gate (1.2 -> 2.4 GHz needs ~3.4 us of
            # sustained matmul activity) inside the first x-DMA's shadow, so
            # the real matmuls start at full clock.
            wsrc = op.tile([128, 128], mybir.dt.bfloat16, name="warmsrc")
            nc.vector.memset(wsrc[:], 0.0)
            warm = ps.tile([128, NFREE], mybir.dt.float32, name="pt0")
            for _ in range(27):
                nc.tensor.matmul(
                    warm[0:64, 0:128],
                    lhsT=wsrc[:, 0:64],
                    rhs=wsrc[:],
                    start=True,
                    stop=True,
                )

            for q in range(QUADS):
                xt = xp.tile([128, QCOLS], mybir.dt.bfloat16)
                if q == 0:
                    # Split the first load: weights + image-half 0 first so
                    # the real matmuls start earlier.
                    nc.sync.dma_start(
                        out=xt[:, 0 : 2 * WCOLS], in_=xd[0, :, 0 : 2 * WCOLS]
                    )
                    c1 = 2 * WCOLS + 2 * HCOLS
                    nc.sync.dma_start(
                        out=xt[:, 2 * WCOLS : c1], in_=xd[0, :, 2 * WCOLS : c1]
                    )
                    nc.sync.dma_start(out=xt[:, c1:QCOLS], in_=xd[0, :, c1:QCOLS])
                else:
                    nc.sync.dma_start(out=xt[:], in_=xd[q])

                tv0 = xt[:, 0:WCOLS].rearrange("p (k m) -> p k m", m=H)
                tv1 = xt[:, WCOLS : 2 * WCOLS].rearrange("p (k m) -> p k m", m=H)
                xb = 2 * WCOLS
                # xh[half][pair] -> [p, n(8), w] view of that image-half
                xh = [
                    [
                        xt[:, xb + (2 * hf + pr) * HCOLS : xb + (2 * hf + pr + 1)
                           * HCOLS].rearrange("p (n w) -> p n w", w=W)
                        for pr in range(2)
                    ]
                    for hf in range(2)
                ]

                pts = [
                    ps.tile([128, NHALF, WO], mybir.dt.float32, name=f"pt{i}")
                    for i in range(4)
                ]  # [pair0 half0, pair0 half1, pair1 half0, pair1 half1]
                # q0 runs half-outer (half1 images arrive late); others run
                # kw-outer so consecutive slots share each tile's lhsT.
                order = (
                    [(kw_, hf) for hf in range(2) for kw_ in range(4)]
                    if q == 0
                    else [(kw_, hf) for kw_ in range(4) for hf in range(2)]
                )
                for kw_i, half in order:
                    kw, xc0, xc1, wo0, wo1 = KW_PLAN[kw_i]
                    if True:
                        rhs0 = xh[half][0][:, :, xc0:xc1]
                        rhs1 = xh[half][1][:, :, xc0:xc1]
                        st = kw == 1
                        sp = kw == 3
                        p0 = pts[half]
                        p1 = pts[2 + half]
                        # NOTE: kw-outer order is load-bearing - consecutive
                        # halves share each tile's lhsT, which lets the PE
                        # stream at ~215 ns/slot.
                        nc.tensor.matmul(
                            p0[0:64, :, wo0:wo1],
                            lhsT=tv0[0:64, kw, :],
                            rhs=rhs0[0:64],
                            start=st,
                            stop=sp,
                            tile_position=(0, 0),
                        )
                        nc.tensor.matmul(
                            p0[64:128, :, wo0:wo1],
                            lhsT=tv0[64:128, kw, :],
                            rhs=rhs0[64:128],
                            start=st,
                            stop=sp,
                            tile_position=(64, 64),
                        )
                        nc.tensor.matmul(
                            p1[64:128, :, wo0:wo1],
                            lhsT=tv1[0:64, kw, :],
                            rhs=rhs1[0:64],
                            start=st,
                            stop=sp,
                            tile_position=(0, 64),
                        )
                        nc.tensor.matmul(
                            p1[0:64, :, wo0:wo1],
                            lhsT=tv1[64:128, kw, :],
                            rhs=rhs1[64:128],
                            start=st,
                            stop=sp,
                            tile_position=(64, 0),
                        )

                ot = op.tile([128, 2, 2, NFREE], mybir.dt.int8)
                if q == QUADS - 1:
                    # Tail: store each psum tile as soon as its copy lands,
                    # spread over both HWDGE rings (inputs are done by now).
                    nc.scalar.copy(ot[:, 0, 0, :], pts[0][:])
                    nc.scalar.dma_start(out=od[q, :, 0, 0], in_=ot[:, 0, 0])
                    nc.vector.tensor_copy(ot[:, 0, 1, :], pts[1][:])
                    nc.sync.dma_start(out=od[q, :, 0, 1], in_=ot[:, 0, 1])
                    nc.scalar.copy(ot[:, 1, 0, :], pts[2][:])
                    nc.scalar.dma_start(out=od[q, :, 1, 0], in_=ot[:, 1, 0])
                    nc.vector.tensor_copy(ot[:, 1, 1, :], pts[3][:])
                    nc.sync.dma_start(out=od[q, :, 1, 1], in_=ot[:, 1, 1])
                else:
                    nc.scalar.copy(ot[:, 0, 0, :], pts[0][:])
                    nc.vector.tensor_copy(ot[:, 0, 1, :], pts[1][:])
                    nc.scalar.copy(ot[:, 1, 0, :], pts[2][:])
                    nc.vector.tensor_copy(ot[:, 1, 1, :], pts[3][:])
                    # SWDGE (gpsimd) keeps store dispatches off the ACT/SP
                    # HWDGE rings, so copies are never head-of-line blocked.
                    nc.gpsimd.dma_start(out=od[q], in_=ot[:])
    nc.compile()
    return nc


def _get_nc():
    global _NC_CACHE
    if _NC_CACHE is None:
        _NC_CACHE = _build_nc()
    return _NC_CACHE


def _prep_x(x):
    """x (16, 512, 64, 64) f32 -> per-core (PAIRS, 128, XCOLS) bf16.

    Partition index p = c'*64 + h for channel pair slot c' in {0, 1};
    free layout [n, w] (no pad columns - kw edges use clipped matmul ranges).
    """
    maps = []
    for k in range(N_CORES):
        xc = x[:, k * CH : (k + 1) * CH]  # (16, 64, 64, 64)
        t = xc.transpose(1, 2, 0, 3)  # (ch, h, n, w)
        maps.append(t.astype(BF16).reshape(PAIRS, 128, XCOLS))
    return maps


def _quant_w(wc):
    """wc (512, 4, 4) -> per-channel psum-scaled weights ws (512, 4, 4) f32
    (psum = conv * sc_c stays in int8 range) and dequant scale dq (512,)."""
    norm = np.sqrt((wc**2).sum(axis=(1, 2)))
    norm = np.maximum(norm, 1e-20)
    sc = 127.0 / (KSAFE * norm)  # psum = conv * sc
    ws = wc * sc[:, None, None]
    dq = (1.0 / sc).astype(np.float32)
    return ws.astype(np.float32), dq


def _prep_t(ws):
    """ws (512, 4, 4) f32 scaled weights -> per-core (PAIRS, 128, WCOLS)
    bf16 banded Toeplitz lhsT blocks.

    lhsT[pair, c'*64 + h, kw*H + ho] = ws[ch, h - ho + 1, kw]
    for 0 <= h - ho + 1 <= 3, ho <= 62 (column 63 stays zero).
    """
    maps = []
    ho = np.arange(HO)
    for k in range(N_CORES):
        wk = ws[k * CH : (k + 1) * CH]  # (64, 4, 4)
        blocks = np.zeros((CH, 4, H, H), dtype=np.float32)  # [ch, kw, h, ho]
        for kh in range(4):
            h = ho + kh - 1
            v = (h >= 0) & (h < H)
            blocks[:, :, h[v], ho[v]] = wk[:, kh, :][:, :, None]
        lt = blocks.transpose(0, 2, 1, 3).reshape(PAIRS, 128, WCOLS)
        maps.append(lt.astype(BF16))
    return maps


def _prep_in(x, ws):
    xs = _prep_x(x)  # (PAIRS, 128, XCOLS) with [n, w] free layout
    ts = _prep_t(ws)  # (PAIRS, 128, WCOLS)
    out = []
    for k in range(N_CORES):
        xq = xs[k].reshape(QUADS, 2, 128, 2, HCOLS)  # [q, pr, p, half, hcols]
        tq = ts[k].reshape(QUADS, 2, 128, WCOLS)
        a = np.empty((QUADS, 128, QCOLS), dtype=BF16)
        a[:, :, 0:WCOLS] = tq[:, 0]
        a[:, :, WCOLS : 2 * WCOLS] = tq[:, 1]
        for hf in range(2):
            for pr in range(2):
                c0 = 2 * WCOLS + (2 * hf + pr) * HCOLS
                a[:, :, c0 : c0 + HCOLS] = xq[:, pr, :, hf, :]
        out.append(np.ascontiguousarray(a))
    return out


def _unswizzle(out_dev, dqk):
    """(PAIRS, 128, 2, NFREE) int8 -> (16, 64, 63, 63) f32 for one core.

    Odd pairs have their two channels partition-swapped (PE quadrant
    wiring); dqk is the per-channel dequant scale (64,) for this core.
    """
    r = out_dev.reshape(QUADS, 2, H, 2, 2, NHALF, WO).astype(np.float32)
    r = r.transpose(0, 3, 1, 2, 4, 5, 6).reshape(PAIRS, 2, H, 2, NHALF, WO)
    # [pair, cslot, ho64, half, n', wo]; odd pairs: cslot = 1 - c'
    r[1::2] = r[1::2, ::-1]
    t = r.transpose(3, 4, 0, 1, 2, 5)  # [half, n', pair, c', ho64, wo]
    out = t.reshape(IMGS, CH, H, WO)[:, :, :HO, :]
    out *= dqk[None, :, None, None]
    return np.ascontiguousarray(out)


def kernel(x, weight, mask, groups=8, stride=1, _trace=False, _trace_kwargs=None):
    global LAST_RESULT
    x = np.ascontiguousarray(np.asarray(x, dtype=np.float32))
    weight = np.asarray(weight, dtype=np.float32)
    mask = np.asarray(mask, dtype=np.float32)

    # Masked weights collapse to one 4x4 filter per output channel.
    wc = (weight * mask).sum(axis=1)  # (512, 4, 4)
    ws, dq = _quant_w(wc)

    ins = _prep_in(x, ws)
    in_maps = [{"xin": ins[k]} for k in range(N_CORES)]

    nc = _get_nc()
    kwargs = {}
    if _trace:
        kwargs["trace"] = True
        if _trace_kwargs:
            kwargs.update(_trace_kwargs)
    res = run_bass_kernel_spmd(nc, in_maps, core_ids=list(range(N_CORES)), **kwargs)
    LAST_RESULT = res

    outs = [
        _unswizzle(res.results[k]["out"], dq[k * CH : (k + 1) * CH])
        for k in range(N_CORES)
    ]
    return np.concatenate(outs, axis=1)


def emulate(x, weight, mask, groups=8, stride=1):
    """Pure-numpy emulation of the device math (same quantization and
    packing) - validates host prep + Toeplitz construction without HW."""
    x = np.asarray(x, dtype=np.float32)
    wc = (np.asarray(weight, np.float32) * np.asarray(mask, np.float32)).sum(axis=1)
    ws, dq = _quant_w(wc)
    ins = _prep_in(x, ws)
    outs = []
    for k in range(N_CORES):
        out_dev = np.zeros((QUADS, 128, 2, 2, NFREE), dtype=np.int8)
        for pair in range(PAIRS):
            q_, pr = pair // 2, pair % 2
            row = ins[k][q_].astype(np.float32)
            xin = np.concatenate(
                [
                    row[:, 2 * WCOLS + pr * HCOLS : 2 * WCOLS + (pr + 1) * HCOLS],
                    row[:, 2 * WCOLS + (2 + pr) * HCOLS : 2 * WCOLS + (3 + pr)
                        * HCOLS],
                ],
                axis=1,
            ).reshape(128, IMGS, W)
            ttb = row[:, pr * WCOLS : (pr + 1) * WCOLS].reshape(128, 4, H)
            swap = pair % 2 == 1
            for half in range(2):
                acc = np.zeros((128, NHALF, WO), dtype=np.float32)
                for kw, xc0, xc1, wo0, wo1 in KW_PLAN:
                    rhs = xin[:, half * NHALF : (half + 1) * NHALF, xc0:xc1]
                    lo = np.einsum("km,knw->mnw", ttb[0:64, kw], rhs[0:64])
                    hi = np.einsum("km,knw->mnw", ttb[64:128, kw], rhs[64:128])
                    if swap:  # odd pairs land partition-swapped
                        acc[64:128, :, wo0:wo1] += lo
                        acc[0:64, :, wo0:wo1] += hi
                    else:
                        acc[0:64, :, wo0:wo1] += lo
                        acc[64:128, :, wo0:wo1] += hi
                q8 = np.clip(np.rint(acc), -128, 127).astype(np.int8)
                out_dev[pair // 2, :, pair % 2, half, :] = q8.reshape(128, NFREE)
        outs.append(_unswizzle(out_dev, dq[k * CH : (k + 1) * CH]))
    return np.concatenate(outs, axis=1)
